# revision 51
# baseline (speedup 1.0000x reference)
"""EnVAE sampling kernel for 8x TRN2 NeuronCores — slot-aligned fused-selection.

Math (per group g, batch element b):
  Xg = X[:, g::8]                                      # (b, 128)
  h  = relu(Xg @ W1[g] + b1[g])                        # (b, 128)
  out= h @ W2[g] + b2[g]; means=out[:, :64]; lv=out[:, 64:]
  z  = means[b, idx] + eps * exp(0.5 * lv[b, idx])

Design (v2, slot-aligned SEGS=2):
  Each group g reads a DISJOINT column slice of X, so the host reorders each
  group's batch independently. The host builds a SEGMENT sequence (latent,
  count) with partial sums C_q constrained to C_q - 128*q in [-128, 0]
  (always feasible: greedy pick with run-splitting). Then every 128-row
  chunk ch of the reordered batch touches only segments {ch, ch+1}, so the
  device's mm2+latent-selection is ONE tiny matmul per chunk with a STATIC
  contiguous moving operand: w2run[:, g, ch:ch+2, :] (the (mean, logvar)
  W2 column pairs for slots ch, ch+1). No per-(chunk,seg) gathered table.

  Device per (group, tile): mm1 fp8 DoubleRow -> psum, relu+bias (ACT/DVE,
  greedy load-balanced, big 1536-col tiles) -> hsb bf16, per-chunk sel
  matmuls -> zt psum, zout DMA'd directly from PSUM as f32 (no vector
  drain). Host finishes: z = zm + b2m + eps * exp(0.5*(zv + b2v)).

  W1 pre-scaled by 16 (fp8 denormals), W2 divided by 16 to compensate.
"""

import numpy as np
import ml_dtypes

import concourse.bass as bass
import concourse.bacc as bacc
import concourse.mybir as mybir
from concourse import bass_utils

OBS = 1024
LAT = 64
G = 8
GS = 128
HID = 128
BATCH = 65536
NCORES = 8
BPC = BATCH // NCORES        # 8192 batch rows per core
CHUNK = 128                  # batch rows per sel chunk (PE stationary width)
NCH = BPC // CHUNK           # 64 chunks per (group, core)
NSLOT = NCH + 1              # 65 W2 table slots per group
ZC = NCH * 4                 # z cols per group = 256 (2 slots x (mean, lv))
W1SCALE = 16.0

# per-group relu tile sizes (sum = BPC). Uniform 1024 with a 3-deep psum
# slot rotation: the psum-recycle chain relu(i) -> mm1(i+3) -> relu(i+3)
# has ~0.55us of turnaround latency; with 3 slots it hides behind ~3 tile
# periods, with 2 it would gate the pipeline (measured: 2-slot 1536-tiles
# run 30% slower despite lower fixed overhead).
SCS_G = {}
for _g in range(G):
    SCS_G[_g] = [1024] * 8
NT = sum(len(v) for v in SCS_G.values())     # 65 tiles
TSTART = {}                  # g -> global index of its first tile
_c = 0
for _g in range(G):
    TSTART[_g] = _c
    _c += len(SCS_G[_g])
TSTART[G] = _c

FP8 = mybir.dt.float8e4
BF16 = mybir.dt.bfloat16
F32 = mybir.dt.float32
NP_FP8 = ml_dtypes.float8_e4m3
NP_BF16 = ml_dtypes.bfloat16

# group n takes columns n, n+8, ... (round-robin)
GROUP_IDX = np.stack([np.arange(n, OBS, G) for n in range(G)])  # (g, gs)

import os
USE_POOL_B1 = os.environ.get("POOL_B1", "1") == "1"
CHAIN_NS = float(os.environ.get("CHAIN", "540"))

SKEW = 5          # sels for tile i emitted after mm1(i+SKEW)
HSBB = 12         # hsb sbuf buffers
XGB = 4           # xg group buffers

# static tile table: (g, t, boff, size, slot); slots rotate 0/1/2
TILES = []
for _g in range(G):
    _off = 0
    for _t, _s in enumerate(SCS_G[_g]):
        TILES.append((_g, _t, _off, _s, len(TILES) % 3))
        _off += _s
    assert _off == BPC

# xt DMA pieces: per group, list of batch-row widths. Ramp-profiled: each
# piece costs a ~650ns HWDGE slot + transfer + 900ns sem, so early pieces
# grow with the consumption curve (2 engines x ~1 tile/1.1us from ~3.5us).
PIECES = {0: [2048, 2048, 4096]}
for _g in range(1, G):
    PIECES[_g] = [4096, 4096]


def build_program(num_devices: int = NCORES):
    nc = bacc.Bacc("TRN2", target_bir_lowering=False, debug=False,
                   num_devices=num_devices)

    # xt[g, p, sc, i, b] = Xg_slot_ordered[sc*1024 + b, p + 64*i]  (fp8)
    xt = nc.dram_tensor("xt", [G, 64, 8, 2, 1024], FP8,
                        kind="ExternalInput").ap()
    # w1[p, g, i, m] = 16 * W1[g, p + 64*i, m]  (fp8)
    w1 = nc.dram_tensor("w1", [64, G, 2, HID], FP8, kind="ExternalInput").ap()
    # w2run[k, g, q, j] = W2[g, k, lat(q) + 64*j] / 16  (bf16)
    w2run = nc.dram_tensor("w2run", [HID, G, NSLOT, 2], BF16,
                           kind="ExternalInput").ap()
    # b1s[k, g] = 16 * b1[g, k]
    b1 = nc.dram_tensor("b1", [HID, G], F32, kind="ExternalInput").ap()
    # zout[g][row, 4*ch + 2*j + m]: j = slot-ch, m: 0=mean 1=logvar (bf16)
    zout = nc.dram_tensor("z", [G, CHUNK, ZC], BF16, kind="ExternalOutput").ap()

    # ---- SBUF / PSUM -----------------------------------------------------
    w1_sb = nc.alloc_sbuf_tensor("w1s", [64, G, 2, HID], FP8).ap()
    b1_sb = nc.alloc_sbuf_tensor("b1s", [HID, G], F32).ap()
    ws_sb = nc.alloc_sbuf_tensor("wss", [HID, G, NSLOT, 2], BF16).ap()
    xg = [nc.alloc_sbuf_tensor(f"xg{k}", [64, 8, 2, 1024], FP8).ap()
          for k in range(XGB)]
    hsb = [nc.alloc_sbuf_tensor(f"hsb{k}", [HID, 1024], BF16).ap()
           for k in range(HSBB)]
    zsb = [nc.alloc_sbuf_tensor(f"zsb{k}", [CHUNK, ZC], BF16).ap()
           for k in range(G)]
    warm = nc.alloc_sbuf_tensor("warm", [1, 1], F32).ap()
    # psum: 3 relu bufs (2 banks each) + 2 zt banks -> a group's sel tile
    # recycles only after zout(g-4), so group-start sels never park PE.SEQ
    hp = [nc.alloc_psum_tensor(f"hp{k}", [HID, 1024], F32).ap()
          for k in range(3)]
    zt = [nc.alloc_psum_tensor(f"zt{k}", [CHUNK, 512], F32).ap()
          for k in range(2)]

    # ---- semaphores ------------------------------------------------------
    # Same-ring DMA completions are FIFO on hardware, so one counting sem
    # per ring-ordered family is safe (CoreSim's reorder check is stricter
    # than the ring contract; it flags these shared counters).
    s_x = nc.alloc_semaphore("s_x")       # SP xt pieces done (+16 each)
    DBG_SEMS = os.environ.get("DBG_SEMS", "0") == "1"
    s_xps = [nc.alloc_semaphore(f"s_xp{k}") for k in range(17)] if DBG_SEMS else None
    s_ws = [nc.alloc_semaphore(f"s_wd{k}") for k in range(3)] if DBG_SEMS else None
    s_zos = [nc.alloc_semaphore(f"s_zod{k}") for k in range(G)] if DBG_SEMS else None
    s_w = nc.alloc_semaphore("s_w")       # w1/ws0/ws1 done (+16, SP ring)
    s_b1 = nc.alloc_semaphore("s_b1")     # b1 DMA done (Pool SWDGE queue)
    s_mm1 = nc.alloc_semaphore("s_mm1")   # mm1 tiles done (PE, +1)
    s_sel = nc.alloc_semaphore("s_sel")   # sel tiles done (PE, +1)
    s_ra = nc.alloc_semaphore("s_ra")     # ACT relus done
    s_rd = nc.alloc_semaphore("s_rd")     # DVE relus done
    s_zo = nc.alloc_semaphore("s_zo")     # zout DMAs done (+16 each)

    # ---- unified ACT/DVE schedule: 64 relus + drains, greedy balance -----
    # ops in emission order; a drain is due once the sels it reads from are
    # emitted. The last group's drain+zout go in two pieces so the final
    # HWDGE round-trip only carries 32 columns.
    # drain key -> (g, lo, hi, sel_need)
    DRAINS = {g: (g, 0, ZC, TSTART[g + 1]) for g in range(G - 1)}
    DRAINS["7a"] = (G - 1, 0, ZC // 2, NT)
    DRAINS["7b"] = (G - 1, ZC // 2, ZC, NT)
    drain_at = {}
    for key, (_g, _lo, _hi, _need) in DRAINS.items():
        drain_at.setdefault(_need - 1 + SKEW, []).append(key)
    # ops MUST mirror the emission order exactly: per-engine semaphore
    # indices are positional, so any divergence deadlocks the device.
    ops = []
    for i in range(NT):
        ops.append(("relu", i))
        for key in drain_at.get(i, []):
            ops.append(("drain", key))
    for j in range(NT - SKEW, NT):
        for key in drain_at.get(j + SKEW, []):
            ops.append(("drain", key))
    for key in DRAINS:
        if ("drain", key) not in ops:
            ops.append(("drain", key))

    def op_cost(e, kind, n):
        return n * 0.8333 + 185.0 if e == "a" else n * 1.0417 + 125.0

    # Schedule-aware engine assignment: simulate start/finish times with the
    # psum +3-slot recycle chain (relu(i) -> mm1(i+3) -> relu(i+3), ~540ns
    # turnaround) and pick the engine that finishes each op earliest. A pure
    # load-greedy leaves ~170ns bubbles whenever one engine gets
    # consecutive slots. LoadActFuncSet (1283ns, auto-inserted on ACT)
    # hides in the pre-data dead zone, so it is NOT seeded.
    CHAIN = CHAIN_NS
    eng_free = {"a": 0.0, "d": 0.0}
    eng_cnt = {"a": 0, "d": 0}
    relu_end = {}
    relu_map = {}
    drain_map = {}
    for kind, v in ops:
        n = TILES[v][3] if kind == "relu" else ZC
        if kind == "relu":
            ready = relu_end.get(v - 3, -CHAIN) + CHAIN
            if v < 2:
                e = "ad"[v]   # ramp: one engine each on the first two tiles
            else:
                e = min(("a", "d"),
                        key=lambda k: (max(eng_free[k], ready)
                                       + op_cost(k, kind, n), eng_free[k]))
            start = max(eng_free[e], ready)
            relu_end[v] = start + op_cost(e, kind, n)
            eng_free[e] = relu_end[v]
        else:
            n = DRAINS[v][2] - DRAINS[v][1]
            if v == "7b" and "7a" in drain_map:
                e = drain_map["7a"][0]
            else:
                e = min(("a", "d"),
                        key=lambda k: eng_free[k] + op_cost(k, kind, n))
            eng_free[e] += op_cost(e, kind, n)
        eng_cnt[e] += 1
        if kind == "relu":
            relu_map[v] = (e, eng_cnt[e])
        else:
            drain_map[v] = (e, eng_cnt[e])

    def relu_sem(i):
        e, idx = relu_map[i]
        return (s_ra if e == "a" else s_rd), idx

    def drain_sem(key):
        e, idx = drain_map[key]
        return (s_ra if e == "a" else s_rd), idx

    # previous occupant of each psum slot (for recycle waits)
    slot_prev = {}
    tile_prev = [None] * NT     # tile index whose relu must finish first
    for i, (_, _, _, _, slot) in enumerate(TILES):
        tile_prev[i] = slot_prev.get(slot)
        slot_prev[slot] = i

    # ---- SP: DMA stream --------------------------------------------------
    # order: w1, g0 pieces + b1 + ws interleaved, g1.., with zout(g) placed
    # after the xt pieces of group g+2 so parking never starves xt supply.
    npiece = [0]

    def emit_xt_piece(g, lo, wdt):
        ins = nc.sync.dma_start(xg[g % XGB][:, lo // 1024:(lo + wdt) // 1024],
                                xt[g, :, lo // 1024:(lo + wdt) // 1024])
        if g >= XGB:
            # slot recycle: all mm1s of group g-XGB consumed xg[g%XGB]
            ins.wait_op(s_mm1, TSTART[g - XGB + 1], "sem-ge")
        ins.then_inc(s_xps[npiece[0]] if DBG_SEMS else s_x, 16)
        npiece[0] += 1

    def emit_zout(key):
        if key == "7full":
            ins = nc.sync.dma_start(zout[G - 1], zsb[G - 1])
            sem, idx = drain_sem("7b")
            ins.wait_op(sem, idx, "sem-ge")
            ins.then_inc(s_zos[G - 1] if DBG_SEMS else s_zo, 16)
            return
        g, lo, hi, _ = DRAINS[key]
        ins = nc.sync.dma_start(zout[g][:, lo:hi], zsb[g][:, lo:hi])
        sem, idx = drain_sem(key)
        ins.wait_op(sem, idx, "sem-ge")
        ins.then_inc(s_zos[g] if DBG_SEMS else s_zo, 16)

    # early SP order tuned for ramp latency: every HWDGE slot costs ~650ns
    # and transfers serialize, so: full w1 first (366ns transfer), then g0 x
    # pieces with the g0-g1 ws slice between them; b1 rides the PARALLEL
    # Pool/SWDGE gen path; the rest of ws goes after g1's x pieces.
    # s_w counts: 16 w1, 32 ws01, 48 ws-rest.
    nc.sync.dma_start(w1_sb, w1).then_inc(s_ws[0] if DBG_SEMS else s_w, 16)
    if USE_POOL_B1:
        nc.gpsimd.dma_start(b1_sb, b1).then_inc(s_b1, 16)
    else:
        nc.sync.dma_start(b1_sb, b1).then_inc(s_b1, 16)
    zout_after = {2: 0, 3: 1, 4: 2, 5: 3, 6: 4, 7: 5}  # g -> zout emitted after
    for g in range(G):
        lo = 0
        for pi, wdt in enumerate(PIECES[g]):
            emit_xt_piece(g, lo, wdt)
            lo += wdt
            if g == 0 and pi == 1:
                nc.sync.dma_start(ws_sb[:, 0:2], w2run[:, 0:2]).then_inc(s_ws[1] if DBG_SEMS else s_w, 32)
        if g == 1:
            nc.sync.dma_start(ws_sb[:, 2:G], w2run[:, 2:G]).then_inc(s_ws[2] if DBG_SEMS else s_w, 32)
        if g in zout_after:
            emit_zout(zout_after[g])
    emit_zout(6)
    emit_zout("7full")

    # map tile -> required s_x count: piece numbers are sequential per group
    # in emission order; tile needs the piece covering boff+size-1.
    pstart = {}
    cnt = 0
    for g in range(G):
        pstart[g] = cnt
        cnt += len(PIECES[g])

    def xt_need(g, boff, size):
        lo = 0
        for pi, wdt in enumerate(PIECES[g]):
            lo += wdt
            if boff + size <= lo:
                return pstart[g] + pi + 1
        raise AssertionError

    # ---- PE stream -------------------------------------------------------
    xneed_max = [0]           # s_x already implied by earlier in-order mm1s

    def emit_mm1(i):
        g, t, boff, size, slot = TILES[i]
        if i == 0:
            nc.tensor.wait_ge(s_ws[0] if DBG_SEMS else s_w, 16)  # w1 loaded
        need = xt_need(g, boff, size)
        need_x = need > xneed_max[0]
        # only one wait fits per instruction; attach the psum-recycle wait
        # (hot on the relu critical path when an engine gets consecutive
        # slots) unless this tile also advances the x-piece requirement, in
        # which case s_x is attached and the recycle wait goes standalone
        # (those tiles sit at piece boundaries where the chain has slack).
        if need_x and tile_prev[i] is not None:
            sem, val = relu_sem(tile_prev[i])
            nc.tensor.wait_ge(sem, val)
        nhalf = size // 512
        for h in range(nhalf):
            ins = nc.tensor.matmul(
                hp[slot][:, 512 * h:512 * h + 512],
                w1_sb[:, g],
                xg[g % XGB][:, t, :, 512 * h:512 * h + 512],
                start=True, stop=True,
                perf_mode=mybir.MatmulPerfMode.DoubleRow)
            if h == 0:
                if need_x:
                    if DBG_SEMS:
                        ins.wait_op(s_xps[need - 1], 16, "sem-ge")
                    else:
                        ins.wait_op(s_x, 16 * need, "sem-ge")
                    xneed_max[0] = need
                elif tile_prev[i] is not None:
                    sem, val = relu_sem(tile_prev[i])
                    ins.wait_op(sem, val, "sem-ge")
            if h == nhalf - 1:
                ins.then_inc(s_mm1, 1)

    def emit_sels(i):
        g, t, boff, size, slot = TILES[i]
        if i == 0:
            if DBG_SEMS:
                nc.tensor.wait_ge(s_ws[1], 32)
            else:
                nc.tensor.wait_ge(s_w, 48)    # w2run g0-g1 slice loaded
        if i == TSTART[2]:
            if DBG_SEMS:
                nc.tensor.wait_ge(s_ws[2], 32)
            else:
                nc.tensor.wait_ge(s_w, 80)    # rest of w2run loaded
        if t == 0 and g >= 4:
            if DBG_SEMS:
                nc.tensor.wait_ge(s_zos[g - 4], 16)
            else:
                nc.tensor.wait_ge(s_zo, 16 * (g - 3))   # zt slot recycled
        nch = size // CHUNK
        ztg = zt[(g // 2) % 2]
        h = 256 * (g % 2)
        for cc in range(nch):
            ch = boff // CHUNK + cc
            ins = nc.tensor.matmul(
                ztg[:, h + 4 * ch:h + 4 * ch + 4],
                hsb[i % HSBB][:, CHUNK * cc:CHUNK * (cc + 1)],
                ws_sb[:, g, ch:ch + 2],
                start=True, stop=True, skip_group_check=True)
            if cc == 0:
                sem, val = relu_sem(i)
                ins.wait_op(sem, val, "sem-ge")
            if cc == nch - 1:
                ins.then_inc(s_sel, 1)

    # ---- ACT / DVE streams ----------------------------------------------
    s_warm = nc.alloc_semaphore("s_warm")
    nc.vector.memset(warm, 0.0).then_inc(s_warm, 1)
    nc.scalar.activation(warm, warm, mybir.ActivationFunctionType.Relu,
                         bias=0.0, scale=1.0).wait_op(s_warm, 1, "sem-ge")

    def emit_relu(i):
        g, t, boff, size, slot = TILES[i]
        eng_act = (relu_map[i][0] == "a")
        o, inp = hsb[i % HSBB][:, :size], hp[slot][:, :size]
        bias = b1_sb[:, g:g + 1]
        eng = nc.scalar if eng_act else nc.vector
        if i < 2:
            eng.wait_ge(s_b1, 16)             # b1 loaded
        if i >= HSBB:
            # hsb slot readers (sels of i-HSBB) must be done
            eng.wait_ge(s_sel, i - HSBB + 1)
        if eng_act:
            ins = nc.scalar.activation(o, inp,
                                       mybir.ActivationFunctionType.Relu,
                                       bias=bias, scale=1.0)
        else:
            ins = nc.vector.tensor_scalar(o, inp, bias, 0.0,
                                          mybir.AluOpType.add,
                                          mybir.AluOpType.max)
        ins.wait_op(s_mm1, i + 1, "sem-ge")
        ins.then_inc(s_ra if eng_act else s_rd, 1)

    def emit_drain(key):
        g, lo, hi, sel_need = DRAINS[key]
        e, _ = drain_map[key]
        base = 256 * (g % 2)
        src = zt[(g // 2) % 2][:, base + lo:base + hi]
        if e == "a":
            ins = nc.scalar.copy(zsb[g][:, lo:hi], src)
        else:
            ins = nc.vector.tensor_copy(zsb[g][:, lo:hi], src)
        ins.wait_op(s_sel, sel_need, "sem-ge")
        ins.then_inc(s_ra if e == "a" else s_rd, 1)

    # ---- interleaved emission (engine-stream order must match `ops`) -----
    emitted_drains = set()
    for i in range(NT):
        emit_mm1(i)
        if i >= SKEW:
            emit_sels(i - SKEW)
        emit_relu(i)
        for key in drain_at.get(i, []):
            emit_drain(key)
            emitted_drains.add(key)
    for j in range(NT - SKEW, NT):
        emit_sels(j)
        for key in drain_at.get(j + SKEW, []):
            emit_drain(key)
            emitted_drains.add(key)
    for key in DRAINS:
        if key not in emitted_drains:
            emit_drain(key)

    nc.compile()
    return nc


# ---------------------------------------------------------------- host side --

def _build_slots(idxg):
    """Greedy band construction: segments (lat, cnt) with partial sums
    C_q in [128q - 128, 128q]. Returns (segs, Q). Always feasible."""
    counts = np.bincount(idxg, minlength=LAT).astype(np.int64)
    rem = counts.copy()
    segs = []
    e = 0                       # C_q - 128*q so far
    total = int(counts.sum())
    while total > 0:
        E = -e                  # window: c in [E, E+128]
        cand = np.where((rem >= E) & (rem <= E + 128) & (rem > 0))[0]
        if len(cand):
            # pick landing closest to mid-band e' = -64  (c* = 64 + E)
            lat = int(cand[np.argmin(np.abs(rem[cand] - (E + 64)))])
            c = int(rem[lat])
        else:
            big = np.where(rem > E + 128)[0]
            assert len(big), (e, rem[rem > 0])
            lat = int(big[0])
            c = E + 64          # split: land mid-band
        segs.append((lat, c))
        rem[lat] -= c
        e = e + c - 128
        assert -128 <= e <= 0, (e, segs)
        total -= c
    assert len(segs) <= NSLOT
    return segs


def _prep_host(X, eps, W1, b1, W2, b2, indices, ncores=NCORES):
    """Per-core input dicts + metadata for unscrambling."""
    W1p = np.ascontiguousarray(
        (W1 * W1SCALE).reshape(G, 2, 64, HID).transpose(2, 0, 1, 3)
    ).astype(NP_FP8)                                   # (64, G, 2, HID)
    b1s = np.ascontiguousarray((W1SCALE * b1).T).astype(np.float32)  # (HID, G)
    W2s = (W2 / W1SCALE).astype(np.float32)            # (G, HID, 128)

    in_maps = []
    metas = []
    for core in range(ncores):
        lo = core * BPC
        xt = np.empty((G, 64, 8, 2, 1024), NP_FP8)
        w2run = np.zeros((HID, G, NSLOT, 2), NP_BF16)
        meta = []
        for g in range(G):
            idxg = np.asarray(indices[g, lo:lo + BPC])
            segs = _build_slots(idxg)
            # stable order rows by latent, then consume per segment
            order_by_lat = np.argsort(idxg, kind="stable")
            lat_start = np.zeros(LAT + 1, np.int64)
            lat_start[1:] = np.cumsum(np.bincount(idxg, minlength=LAT))
            taken = np.zeros(LAT, np.int64)
            order = np.empty(BPC, np.int64)
            slot_of_pos = np.empty(BPC, np.int64)
            lat_of_pos = np.empty(BPC, np.int64)
            p = 0
            for q, (lat, c) in enumerate(segs):
                s0 = lat_start[lat] + taken[lat]
                order[p:p + c] = order_by_lat[s0:s0 + c]
                slot_of_pos[p:p + c] = q
                lat_of_pos[p:p + c] = lat
                taken[lat] += c
                p += c
                w2run[:, g, q, 0] = W2s[g][:, lat]
                w2run[:, g, q, 1] = W2s[g][:, LAT + lat]
            assert p == BPC
            ch_of_pos = np.arange(BPC) // CHUNK
            j_of_pos = slot_of_pos - ch_of_pos
            assert j_of_pos.min() >= 0 and j_of_pos.max() <= 1
            Xg = X[lo + order][:, GROUP_IDX[g]].astype(NP_FP8)  # (BPC, 128)
            # pack [p, sc, i, b]: col k = p + 64*i
            xt[g] = Xg.reshape(8, 1024, 2, 64).transpose(3, 0, 2, 1)
            meta.append((order, lat_of_pos, j_of_pos))
        in_maps.append({"xt": xt, "w1": W1p, "w2run": w2run, "b1": b1s})
        metas.append(meta)
    return in_maps, metas


def _finish_host(zdev, meta, eps_c, b2):
    """zdev: (G, CHUNK, ZC) f32; returns z (G, BPC) in original batch order."""
    z = np.empty((G, BPC), np.float32)
    pos = np.arange(BPC)
    rows = pos % CHUNK
    ch = pos // CHUNK
    for g in range(G):
        order, lat_of_pos, j_of_pos = meta[g]
        col = 4 * ch + 2 * j_of_pos
        zm = zdev[g][rows, col]
        zv = zdev[g][rows, col + 1]
        zs = (zm + b2[g, lat_of_pos] +
              eps_c[g, order] * np.exp(0.5 * (zv + b2[g, LAT + lat_of_pos])))
        z[g, order] = zs
    return z


_NC_CACHE = {}


def kernel(X, eps, W1, b1, W2, b2, indices):
    if "nc" not in _NC_CACHE:
        _NC_CACHE["nc"] = build_program(NCORES)
    nc = _NC_CACHE["nc"]
    in_maps, metas = _prep_host(X, eps, W1, b1, W2, b2, indices)
    res = bass_utils.run_bass_kernel_spmd(nc, in_maps,
                                          core_ids=list(range(NCORES)))
    z = np.zeros((G, BATCH), np.float32)
    for core in range(NCORES):
        lo = core * BPC
        zdev = np.asarray(res.results[core]["z"]).astype(np.float32)
        z[:, lo:lo + BPC] = _finish_host(zdev, metas[core],
                                         np.asarray(eps)[:, lo:lo + BPC],
                                         np.asarray(b2))
    return z.astype(np.float32)


# revision 53
# speedup vs baseline: 1.0026x; 1.0026x over previous
"""EnVAE sampling kernel for 8x TRN2 NeuronCores — slot-aligned fused-selection.

Math (per group g, batch element b):
  Xg = X[:, g::8]                                      # (b, 128)
  h  = relu(Xg @ W1[g] + b1[g])                        # (b, 128)
  out= h @ W2[g] + b2[g]; means=out[:, :64]; lv=out[:, 64:]
  z  = means[b, idx] + eps * exp(0.5 * lv[b, idx])

Design (v2, slot-aligned SEGS=2):
  Each group g reads a DISJOINT column slice of X, so the host reorders each
  group's batch independently. The host builds a SEGMENT sequence (latent,
  count) with partial sums C_q constrained to C_q - 128*q in [-128, 0]
  (always feasible: greedy pick with run-splitting). Then every 128-row
  chunk ch of the reordered batch touches only segments {ch, ch+1}, so the
  device's mm2+latent-selection is ONE tiny matmul per chunk with a STATIC
  contiguous moving operand: w2run[:, g, ch:ch+2, :] (the (mean, logvar)
  W2 column pairs for slots ch, ch+1). No per-(chunk,seg) gathered table.

  Device per (group, tile): mm1 fp8 DoubleRow -> psum, relu+bias (ACT/DVE,
  greedy load-balanced, big 1536-col tiles) -> hsb bf16, per-chunk sel
  matmuls -> zt psum, zout DMA'd directly from PSUM as f32 (no vector
  drain). Host finishes: z = zm + b2m + eps * exp(0.5*(zv + b2v)).

  W1 pre-scaled by 16 (fp8 denormals), W2 divided by 16 to compensate.
"""

import numpy as np
import ml_dtypes

import concourse.bass as bass
import concourse.bacc as bacc
import concourse.mybir as mybir
from concourse import bass_utils

OBS = 1024
LAT = 64
G = 8
GS = 128
HID = 128
BATCH = 65536
NCORES = 8
BPC = BATCH // NCORES        # 8192 batch rows per core
CHUNK = 128                  # batch rows per sel chunk (PE stationary width)
NCH = BPC // CHUNK           # 64 chunks per (group, core)
NSLOT = NCH + 1              # 65 W2 table slots per group
ZC = NCH * 4                 # z cols per group = 256 (2 slots x (mean, lv))
W1SCALE = 16.0

# per-group relu tile sizes (sum = BPC). Uniform 1024 with a 3-deep psum
# slot rotation: the psum-recycle chain relu(i) -> mm1(i+3) -> relu(i+3)
# has ~0.55us of turnaround latency; with 3 slots it hides behind ~3 tile
# periods, with 2 it would gate the pipeline (measured: 2-slot 1536-tiles
# run 30% slower despite lower fixed overhead).
SCS_G = {}
for _g in range(G):
    SCS_G[_g] = [1024] * 8
NT = sum(len(v) for v in SCS_G.values())     # 65 tiles
TSTART = {}                  # g -> global index of its first tile
_c = 0
for _g in range(G):
    TSTART[_g] = _c
    _c += len(SCS_G[_g])
TSTART[G] = _c

FP8 = mybir.dt.float8e4
BF16 = mybir.dt.bfloat16
F32 = mybir.dt.float32
NP_FP8 = ml_dtypes.float8_e4m3
NP_BF16 = ml_dtypes.bfloat16

# group n takes columns n, n+8, ... (round-robin)
GROUP_IDX = np.stack([np.arange(n, OBS, G) for n in range(G)])  # (g, gs)

import os
USE_POOL_B1 = os.environ.get("POOL_B1", "1") == "1"
CHAIN_NS = float(os.environ.get("CHAIN", "540"))

SKEW = 5          # sels for tile i emitted after mm1(i+SKEW)
HSBB = 12         # hsb sbuf buffers
XGB = 4           # xg group buffers

# static tile table: (g, t, boff, size, slot); slots rotate 0/1/2
TILES = []
for _g in range(G):
    _off = 0
    for _t, _s in enumerate(SCS_G[_g]):
        TILES.append((_g, _t, _off, _s, len(TILES) % 3))
        _off += _s
    assert _off == BPC

# xt DMA pieces: per group, list of batch-row widths. Ramp-profiled: each
# piece costs a ~650ns HWDGE slot + transfer + 900ns sem, so early pieces
# grow with the consumption curve (2 engines x ~1 tile/1.1us from ~3.5us).
PIECES = {0: [2048, 2048, 4096]}
for _g in range(1, G):
    PIECES[_g] = [4096, 4096]


def build_program(num_devices: int = NCORES):
    nc = bacc.Bacc("TRN2", target_bir_lowering=False, debug=False,
                   num_devices=num_devices)

    # xt[g, p, sc, i, b] = Xg_slot_ordered[sc*1024 + b, p + 64*i]  (fp8)
    xt = nc.dram_tensor("xt", [G, 64, 8, 2, 1024], FP8,
                        kind="ExternalInput").ap()
    # w1[p, g, i, m] = 16 * W1[g, p + 64*i, m]  (fp8)
    w1 = nc.dram_tensor("w1", [64, G, 2, HID], FP8, kind="ExternalInput").ap()
    # w2run[k, g, q, j] = W2[g, k, lat(q) + 64*j] / 16  (bf16)
    w2run = nc.dram_tensor("w2run", [HID, G, NSLOT, 2], BF16,
                           kind="ExternalInput").ap()
    # b1s[k, g] = 16 * b1[g, k]
    b1 = nc.dram_tensor("b1", [HID, G], F32, kind="ExternalInput").ap()
    # zout[g][row, 4*ch + 2*j + m]: j = slot-ch, m: 0=mean 1=logvar (bf16)
    zout = nc.dram_tensor("z", [G, CHUNK, ZC], BF16, kind="ExternalOutput").ap()

    # ---- SBUF / PSUM -----------------------------------------------------
    w1_sb = nc.alloc_sbuf_tensor("w1s", [64, G, 2, HID], FP8).ap()
    b1_sb = nc.alloc_sbuf_tensor("b1s", [HID, G], F32).ap()
    ws_sb = nc.alloc_sbuf_tensor("wss", [HID, G, NSLOT, 2], BF16).ap()
    xg = [nc.alloc_sbuf_tensor(f"xg{k}", [64, 8, 2, 1024], FP8).ap()
          for k in range(XGB)]
    hsb = [nc.alloc_sbuf_tensor(f"hsb{k}", [HID, 1024], BF16).ap()
           for k in range(HSBB)]
    zsb = [nc.alloc_sbuf_tensor(f"zsb{k}", [CHUNK, ZC], BF16).ap()
           for k in range(G)]
    warm = nc.alloc_sbuf_tensor("warm", [1, 1], F32).ap()
    # psum: 3 relu bufs (2 banks each) + 2 zt banks -> a group's sel tile
    # recycles only after zout(g-4), so group-start sels never park PE.SEQ
    hp = [nc.alloc_psum_tensor(f"hp{k}", [HID, 1024], F32).ap()
          for k in range(3)]
    zt = [nc.alloc_psum_tensor(f"zt{k}", [CHUNK, 512], F32).ap()
          for k in range(2)]

    # ---- semaphores ------------------------------------------------------
    # Same-ring DMA completions are FIFO on hardware, so one counting sem
    # per ring-ordered family is safe (CoreSim's reorder check is stricter
    # than the ring contract; it flags these shared counters).
    s_x = nc.alloc_semaphore("s_x")       # SP xt pieces done (+16 each)
    DBG_SEMS = os.environ.get("DBG_SEMS", "0") == "1"
    s_xps = [nc.alloc_semaphore(f"s_xp{k}") for k in range(17)] if DBG_SEMS else None
    s_ws = [nc.alloc_semaphore(f"s_wd{k}") for k in range(3)] if DBG_SEMS else None
    s_zos = [nc.alloc_semaphore(f"s_zod{k}") for k in range(G)] if DBG_SEMS else None
    s_w = nc.alloc_semaphore("s_w")       # w1/ws0/ws1 done (+16, SP ring)
    s_b1 = nc.alloc_semaphore("s_b1")     # b1 DMA done (Pool SWDGE queue)
    s_mm1 = nc.alloc_semaphore("s_mm1")   # mm1 tiles done (PE, +1)
    s_sel = nc.alloc_semaphore("s_sel")   # sel tiles done (PE, +1)
    s_ra = nc.alloc_semaphore("s_ra")     # ACT relus done
    s_rd = nc.alloc_semaphore("s_rd")     # DVE relus done
    s_zo = nc.alloc_semaphore("s_zo")     # zout DMAs done (+16 each)

    # ---- unified ACT/DVE schedule: 64 relus + drains, greedy balance -----
    # ops in emission order; a drain is due once the sels it reads from are
    # emitted. The last group's drain+zout go in two pieces so the final
    # HWDGE round-trip only carries 32 columns.
    # drain key -> (g, lo, hi, sel_need)
    DRAINS = {g: (g, 0, ZC, TSTART[g + 1]) for g in range(G)}
    drain_at = {}
    for key, (_g, _lo, _hi, _need) in DRAINS.items():
        drain_at.setdefault(_need - 1 + SKEW, []).append(key)
    # ops MUST mirror the emission order exactly: per-engine semaphore
    # indices are positional, so any divergence deadlocks the device.
    ops = []
    for i in range(NT):
        ops.append(("relu", i))
        for key in drain_at.get(i, []):
            ops.append(("drain", key))
    for j in range(NT - SKEW, NT):
        for key in drain_at.get(j + SKEW, []):
            ops.append(("drain", key))
    for key in DRAINS:
        if ("drain", key) not in ops:
            ops.append(("drain", key))

    def op_cost(e, kind, n):
        return n * 0.8333 + 185.0 if e == "a" else n * 1.0417 + 125.0

    # Schedule-aware engine assignment: simulate start/finish times with the
    # psum +3-slot recycle chain (relu(i) -> mm1(i+3) -> relu(i+3), ~540ns
    # turnaround) and pick the engine that finishes each op earliest. A pure
    # load-greedy leaves ~170ns bubbles whenever one engine gets
    # consecutive slots. LoadActFuncSet (1283ns, auto-inserted on ACT)
    # hides in the pre-data dead zone, so it is NOT seeded.
    CHAIN = CHAIN_NS
    eng_free = {"a": 0.0, "d": 0.0}
    eng_cnt = {"a": 0, "d": 0}
    relu_end = {}
    relu_map = {}
    drain_map = {}
    for kind, v in ops:
        n = TILES[v][3] if kind == "relu" else ZC
        if kind == "relu":
            ready = relu_end.get(v - 3, -CHAIN) + CHAIN
            if v < 2:
                e = "ad"[v]   # ramp: one engine each on the first two tiles
            else:
                e = min(("a", "d"),
                        key=lambda k: (max(eng_free[k], ready)
                                       + op_cost(k, kind, n), eng_free[k]))
            start = max(eng_free[e], ready)
            relu_end[v] = start + op_cost(e, kind, n)
            eng_free[e] = relu_end[v]
        else:
            n = DRAINS[v][2] - DRAINS[v][1]
            e = min(("a", "d"),
                    key=lambda k: eng_free[k] + op_cost(k, kind, n))
            eng_free[e] += op_cost(e, kind, n)
        eng_cnt[e] += 1
        if kind == "relu":
            relu_map[v] = (e, eng_cnt[e])
        else:
            drain_map[v] = (e, eng_cnt[e])

    def relu_sem(i):
        e, idx = relu_map[i]
        return (s_ra if e == "a" else s_rd), idx

    def drain_sem(key):
        e, idx = drain_map[key]
        return (s_ra if e == "a" else s_rd), idx

    # previous occupant of each psum slot (for recycle waits)
    slot_prev = {}
    tile_prev = [None] * NT     # tile index whose relu must finish first
    for i, (_, _, _, _, slot) in enumerate(TILES):
        tile_prev[i] = slot_prev.get(slot)
        slot_prev[slot] = i

    # ---- SP: DMA stream --------------------------------------------------
    # order: w1, g0 pieces + b1 + ws interleaved, g1.., with zout(g) placed
    # after the xt pieces of group g+2 so parking never starves xt supply.
    npiece = [0]

    def emit_xt_piece(g, lo, wdt):
        ins = nc.sync.dma_start(xg[g % XGB][:, lo // 1024:(lo + wdt) // 1024],
                                xt[g, :, lo // 1024:(lo + wdt) // 1024])
        if g >= XGB:
            # slot recycle: all mm1s of group g-XGB consumed xg[g%XGB]
            ins.wait_op(s_mm1, TSTART[g - XGB + 1], "sem-ge")
        ins.then_inc(s_xps[npiece[0]] if DBG_SEMS else s_x, 16)
        npiece[0] += 1

    def emit_zout(key):
        g, lo, hi, _ = DRAINS[key]
        ins = nc.sync.dma_start(zout[g][:, lo:hi], zsb[g][:, lo:hi])
        sem, idx = drain_sem(key)
        ins.wait_op(sem, idx, "sem-ge")
        ins.then_inc(s_zos[g] if DBG_SEMS else s_zo, 16)

    # early SP order tuned for ramp latency: every HWDGE slot costs ~650ns
    # and transfers serialize, so: full w1 first (366ns transfer), then g0 x
    # pieces with the g0-g1 ws slice between them; b1 rides the PARALLEL
    # Pool/SWDGE gen path; the rest of ws goes after g1's x pieces.
    # s_w counts: 16 w1, 32 ws01, 48 ws-rest.
    nc.sync.dma_start(w1_sb, w1).then_inc(s_ws[0] if DBG_SEMS else s_w, 16)
    if USE_POOL_B1:
        nc.gpsimd.dma_start(b1_sb, b1).then_inc(s_b1, 16)
    else:
        nc.sync.dma_start(b1_sb, b1).then_inc(s_b1, 16)
    zout_after = {2: 0, 3: 1, 4: 2, 5: 3, 6: 4, 7: 5}  # g -> zout emitted after
    for g in range(G):
        lo = 0
        for pi, wdt in enumerate(PIECES[g]):
            emit_xt_piece(g, lo, wdt)
            lo += wdt
            if g == 0 and pi == 1:
                nc.sync.dma_start(ws_sb[:, 0:2], w2run[:, 0:2]).then_inc(s_ws[1] if DBG_SEMS else s_w, 32)
        if g == 1:
            nc.sync.dma_start(ws_sb[:, 2:G], w2run[:, 2:G]).then_inc(s_ws[2] if DBG_SEMS else s_w, 32)
        if g in zout_after:
            emit_zout(zout_after[g])
    emit_zout(6)
    emit_zout(7)

    # map tile -> required s_x count: piece numbers are sequential per group
    # in emission order; tile needs the piece covering boff+size-1.
    pstart = {}
    cnt = 0
    for g in range(G):
        pstart[g] = cnt
        cnt += len(PIECES[g])

    def xt_need(g, boff, size):
        lo = 0
        for pi, wdt in enumerate(PIECES[g]):
            lo += wdt
            if boff + size <= lo:
                return pstart[g] + pi + 1
        raise AssertionError

    # ---- PE stream -------------------------------------------------------
    xneed_max = [0]           # s_x already implied by earlier in-order mm1s

    def emit_mm1(i):
        g, t, boff, size, slot = TILES[i]
        if i == 0:
            nc.tensor.wait_ge(s_ws[0] if DBG_SEMS else s_w, 16)  # w1 loaded
        need = xt_need(g, boff, size)
        need_x = need > xneed_max[0]
        # only one wait fits per instruction; attach the psum-recycle wait
        # (hot on the relu critical path when an engine gets consecutive
        # slots) unless this tile also advances the x-piece requirement, in
        # which case s_x is attached and the recycle wait goes standalone
        # (those tiles sit at piece boundaries where the chain has slack).
        if need_x and tile_prev[i] is not None:
            sem, val = relu_sem(tile_prev[i])
            nc.tensor.wait_ge(sem, val)
        nhalf = size // 512
        for h in range(nhalf):
            ins = nc.tensor.matmul(
                hp[slot][:, 512 * h:512 * h + 512],
                w1_sb[:, g],
                xg[g % XGB][:, t, :, 512 * h:512 * h + 512],
                start=True, stop=True,
                perf_mode=mybir.MatmulPerfMode.DoubleRow)
            if h == 0:
                if need_x:
                    if DBG_SEMS:
                        ins.wait_op(s_xps[need - 1], 16, "sem-ge")
                    else:
                        ins.wait_op(s_x, 16 * need, "sem-ge")
                    xneed_max[0] = need
                elif tile_prev[i] is not None:
                    sem, val = relu_sem(tile_prev[i])
                    ins.wait_op(sem, val, "sem-ge")
            if h == nhalf - 1:
                ins.then_inc(s_mm1, 1)

    def emit_sels(i):
        g, t, boff, size, slot = TILES[i]
        if i == 0:
            if DBG_SEMS:
                nc.tensor.wait_ge(s_ws[1], 32)
            else:
                nc.tensor.wait_ge(s_w, 48)    # w2run g0-g1 slice loaded
        if i == TSTART[2]:
            if DBG_SEMS:
                nc.tensor.wait_ge(s_ws[2], 32)
            else:
                nc.tensor.wait_ge(s_w, 80)    # rest of w2run loaded
        if t == 0 and g >= 4:
            if DBG_SEMS:
                nc.tensor.wait_ge(s_zos[g - 4], 16)
            else:
                nc.tensor.wait_ge(s_zo, 16 * (g - 3))   # zt slot recycled
        nch = size // CHUNK
        ztg = zt[(g // 2) % 2]
        h = 256 * (g % 2)
        for cc in range(nch):
            ch = boff // CHUNK + cc
            ins = nc.tensor.matmul(
                ztg[:, h + 4 * ch:h + 4 * ch + 4],
                hsb[i % HSBB][:, CHUNK * cc:CHUNK * (cc + 1)],
                ws_sb[:, g, ch:ch + 2],
                start=True, stop=True, skip_group_check=True)
            if cc == 0:
                sem, val = relu_sem(i)
                ins.wait_op(sem, val, "sem-ge")
            if cc == nch - 1:
                ins.then_inc(s_sel, 1)

    # ---- ACT / DVE streams ----------------------------------------------
    s_warm = nc.alloc_semaphore("s_warm")
    nc.vector.memset(warm, 0.0).then_inc(s_warm, 1)
    nc.scalar.activation(warm, warm, mybir.ActivationFunctionType.Relu,
                         bias=0.0, scale=1.0).wait_op(s_warm, 1, "sem-ge")

    def emit_relu(i):
        g, t, boff, size, slot = TILES[i]
        eng_act = (relu_map[i][0] == "a")
        o, inp = hsb[i % HSBB][:, :size], hp[slot][:, :size]
        bias = b1_sb[:, g:g + 1]
        eng = nc.scalar if eng_act else nc.vector
        if i < 2:
            eng.wait_ge(s_b1, 16)             # b1 loaded
        if i >= HSBB:
            # hsb slot readers (sels of i-HSBB) must be done
            eng.wait_ge(s_sel, i - HSBB + 1)
        if eng_act:
            ins = nc.scalar.activation(o, inp,
                                       mybir.ActivationFunctionType.Relu,
                                       bias=bias, scale=1.0)
        else:
            ins = nc.vector.tensor_scalar(o, inp, bias, 0.0,
                                          mybir.AluOpType.add,
                                          mybir.AluOpType.max)
        ins.wait_op(s_mm1, i + 1, "sem-ge")
        ins.then_inc(s_ra if eng_act else s_rd, 1)

    def emit_drain(key):
        g, lo, hi, sel_need = DRAINS[key]
        e, _ = drain_map[key]
        base = 256 * (g % 2)
        src = zt[(g // 2) % 2][:, base + lo:base + hi]
        if e == "a":
            ins = nc.scalar.copy(zsb[g][:, lo:hi], src)
        else:
            ins = nc.vector.tensor_copy(zsb[g][:, lo:hi], src)
        ins.wait_op(s_sel, sel_need, "sem-ge")
        ins.then_inc(s_ra if e == "a" else s_rd, 1)

    # ---- interleaved emission (engine-stream order must match `ops`) -----
    emitted_drains = set()
    for i in range(NT):
        emit_mm1(i)
        if i >= SKEW:
            emit_sels(i - SKEW)
        emit_relu(i)
        for key in drain_at.get(i, []):
            emit_drain(key)
            emitted_drains.add(key)
    for j in range(NT - SKEW, NT):
        emit_sels(j)
        for key in drain_at.get(j + SKEW, []):
            emit_drain(key)
            emitted_drains.add(key)
    for key in DRAINS:
        if key not in emitted_drains:
            emit_drain(key)

    nc.compile()
    return nc


# ---------------------------------------------------------------- host side --

def _build_slots(idxg):
    """Greedy band construction: segments (lat, cnt) with partial sums
    C_q in [128q - 128, 128q]. Returns (segs, Q). Always feasible."""
    counts = np.bincount(idxg, minlength=LAT).astype(np.int64)
    rem = counts.copy()
    segs = []
    e = 0                       # C_q - 128*q so far
    total = int(counts.sum())
    while total > 0:
        E = -e                  # window: c in [E, E+128]
        cand = np.where((rem >= E) & (rem <= E + 128) & (rem > 0))[0]
        if len(cand):
            # pick landing closest to mid-band e' = -64  (c* = 64 + E)
            lat = int(cand[np.argmin(np.abs(rem[cand] - (E + 64)))])
            c = int(rem[lat])
        else:
            big = np.where(rem > E + 128)[0]
            assert len(big), (e, rem[rem > 0])
            lat = int(big[0])
            c = E + 64          # split: land mid-band
        segs.append((lat, c))
        rem[lat] -= c
        e = e + c - 128
        assert -128 <= e <= 0, (e, segs)
        total -= c
    assert len(segs) <= NSLOT
    return segs


def _prep_host(X, eps, W1, b1, W2, b2, indices, ncores=NCORES):
    """Per-core input dicts + metadata for unscrambling."""
    W1p = np.ascontiguousarray(
        (W1 * W1SCALE).reshape(G, 2, 64, HID).transpose(2, 0, 1, 3)
    ).astype(NP_FP8)                                   # (64, G, 2, HID)
    b1s = np.ascontiguousarray((W1SCALE * b1).T).astype(np.float32)  # (HID, G)
    W2s = (W2 / W1SCALE).astype(np.float32)            # (G, HID, 128)

    in_maps = []
    metas = []
    for core in range(ncores):
        lo = core * BPC
        xt = np.empty((G, 64, 8, 2, 1024), NP_FP8)
        w2run = np.zeros((HID, G, NSLOT, 2), NP_BF16)
        meta = []
        for g in range(G):
            idxg = np.asarray(indices[g, lo:lo + BPC])
            segs = _build_slots(idxg)
            # stable order rows by latent, then consume per segment
            order_by_lat = np.argsort(idxg, kind="stable")
            lat_start = np.zeros(LAT + 1, np.int64)
            lat_start[1:] = np.cumsum(np.bincount(idxg, minlength=LAT))
            taken = np.zeros(LAT, np.int64)
            order = np.empty(BPC, np.int64)
            slot_of_pos = np.empty(BPC, np.int64)
            lat_of_pos = np.empty(BPC, np.int64)
            p = 0
            for q, (lat, c) in enumerate(segs):
                s0 = lat_start[lat] + taken[lat]
                order[p:p + c] = order_by_lat[s0:s0 + c]
                slot_of_pos[p:p + c] = q
                lat_of_pos[p:p + c] = lat
                taken[lat] += c
                p += c
                w2run[:, g, q, 0] = W2s[g][:, lat]
                w2run[:, g, q, 1] = W2s[g][:, LAT + lat]
            assert p == BPC
            ch_of_pos = np.arange(BPC) // CHUNK
            j_of_pos = slot_of_pos - ch_of_pos
            assert j_of_pos.min() >= 0 and j_of_pos.max() <= 1
            Xg = X[lo + order][:, GROUP_IDX[g]].astype(NP_FP8)  # (BPC, 128)
            # pack [p, sc, i, b]: col k = p + 64*i
            xt[g] = Xg.reshape(8, 1024, 2, 64).transpose(3, 0, 2, 1)
            meta.append((order, lat_of_pos, j_of_pos))
        in_maps.append({"xt": xt, "w1": W1p, "w2run": w2run, "b1": b1s})
        metas.append(meta)
    return in_maps, metas


def _finish_host(zdev, meta, eps_c, b2):
    """zdev: (G, CHUNK, ZC) f32; returns z (G, BPC) in original batch order."""
    z = np.empty((G, BPC), np.float32)
    pos = np.arange(BPC)
    rows = pos % CHUNK
    ch = pos // CHUNK
    for g in range(G):
        order, lat_of_pos, j_of_pos = meta[g]
        col = 4 * ch + 2 * j_of_pos
        zm = zdev[g][rows, col]
        zv = zdev[g][rows, col + 1]
        zs = (zm + b2[g, lat_of_pos] +
              eps_c[g, order] * np.exp(0.5 * (zv + b2[g, LAT + lat_of_pos])))
        z[g, order] = zs
    return z


_NC_CACHE = {}


def kernel(X, eps, W1, b1, W2, b2, indices):
    if "nc" not in _NC_CACHE:
        _NC_CACHE["nc"] = build_program(NCORES)
    nc = _NC_CACHE["nc"]
    in_maps, metas = _prep_host(X, eps, W1, b1, W2, b2, indices)
    res = bass_utils.run_bass_kernel_spmd(nc, in_maps,
                                          core_ids=list(range(NCORES)))
    z = np.zeros((G, BATCH), np.float32)
    for core in range(NCORES):
        lo = core * BPC
        zdev = np.asarray(res.results[core]["z"]).astype(np.float32)
        z[:, lo:lo + BPC] = _finish_host(zdev, metas[core],
                                         np.asarray(eps)[:, lo:lo + BPC],
                                         np.asarray(b2))
    return z.astype(np.float32)
